# revision 2
# baseline (speedup 1.0000x reference)
"""Multi-head causal attention (B=4, S=2048, D=1024, H=16) on 8 NeuronCores, v2.

Sharding: core c handles batch b = c//2 and head-group g = c%2 (8 heads).
Host sums the two partial output projections per batch and adds bo.

v2 on-chip design (per core), built around the TRN2 cost model:
  - All projections run as fp8e4 DoubleRow matmuls (0.5 cyc/row, 256-deep
    contraction chunks).  Weights are host-scaled by 32 (fp8 subnormal
    avoidance) and split W = Wh + Wl; X = Xh + Xl likewise.  3-term
    products (Wh*Xh + Wl*Xh + Wh*Xl) recover ~fp16 precision at 0.75x
    the fp16 PE cost.
  - Scores: Q evacuated to fp8 (q8), K to a block (Kh|Kl) fp8 pair; one
    DoubleRow matmul per (head, block) computes q8*(Kh+Kl) via a
    stride-0 moving slot dim: 0.5x fp16 cost.
  - exp on ACT (fp32 psum -> fp16 et), scale 0.125/1024 folds the
    weight scaling out.  Causal diagonal blocks masked by a DVE
    multiply (both heads in one op).
  - PV is flipped: out [sq=128, 65] with et stationary and V moving
    (64 V cols + one 32.0 column that emits 32*rowsum, cancelling the
    32x V scale in the normalize divide).  Full 128 output partitions
    -> ~0.5x the unflipped fp16 cost.
  - Normalize: DVE tensor_scalar divide by the per-partition rowsum
    column, direct to fp16 A tiles [sq, 2*64].
  - A transposed per 128-block on the PE (fp16 identity transpose),
    evacuated by GPSIMD (Pool) to at16[g, s]; O-projection in fp16,
    Pool-evacuated to fp16 and DMA'd out.
  - Pipelining: V-projection chunks and the next pair's Q/K projections
    are interleaved into the attention phase as PE filler so the PE
    never waits on the ACT exp backlog.
  - PSUM: 8 banks exactly — qk [128,2,512]x2 (4), shared proj/O/transpose
    ring x2 (2), PV chains x2 (2).

Walrus wait-slot legality: Tile's wait assigner can emit >1 sem wait per
engine instruction; extras are split onto same-engine NoOps.
"""

import sys

for _p in ("/opt/trn_rl_repo",):
    if _p not in sys.path:
        sys.path.insert(0, _p)

from contextlib import ExitStack

import numpy as np
import ml_dtypes

import concourse.bass as bass
import concourse.mybir as mybir
import concourse.tile as tile
from concourse.bass_utils import run_bass_kernel_spmd

import bass_rust

F8 = mybir.dt.float8e4
F16 = mybir.dt.float16
F32 = mybir.dt.float32
AF = mybir.ActivationFunctionType
DR = mybir.MatmulPerfMode.DoubleRow
NF8 = ml_dtypes.float8_e4m3

B, S, D, H = 4, 2048, 1024, 16
HD = D // H  # 64
GH = 8  # heads per group
GW = GH * HD  # 512 columns per group
WS = 32.0  # host-side weight prescale (fp8 subnormal avoidance)
FILL_FULL = 0.4   # PE-filler credit per full score chunk
FILL_DIAG = 0.2   # per diagonal (windowed) chunk
EXP_SCALE = 0.125 / (WS * WS)

_SPLITTABLE = {
    "InstMatmult", "InstLdweights", "InstActivation", "InstTensorCopy",
    "InstTensorTensor", "InstTensorScalarPtr", "InstTensorReduce",
    "InstMemset", "InstDMACopy", "InstReciprocal", "InstIota",
    "InstTensorTensorReduce", "InstBNStats", "InstBNStatsAggregate",
    "InstStreamShuffle", "InstNoOp", "InstPool", "InstMax", "InstDrain",
    "InstDmaTransposeAnt",
}


def _legalize_waits(nc, max_waits=1):
    """Walrus codegen accepts at most one sync-wait command per engine
    instruction; split extras onto same-engine NoOps inserted immediately
    before (the engine blocks at the same program point)."""
    ctr = 0
    for fn in nc.m.functions:
        for blk in fn.blocks:
            out = []
            for ins in blk.instructions:
                si = ins.sync_info
                if (
                    si is not None
                    and len(si.on_wait) > max_waits
                    and type(ins).__name__ in _SPLITTABLE
                ):
                    waits = list(si.on_wait)
                    extra, keep = waits[:-max_waits], waits[-max_waits:]
                    for w in extra:
                        nop = mybir.InstNoOp(name=f"waitnop-{ctr}", ins=[], outs=[])
                        ctr += 1
                        nop.engine = ins.engine
                        nop.sync_info = bass_rust.SyncInfo(on_wait=[w], on_update=[])
                        out.append(nop)
                    ins.sync_info = bass_rust.SyncInfo(
                        on_wait=keep, on_update=list(si.on_update)
                    )
                out.append(ins)
            blk.instructions[:] = out
    return ctr


def _stride0_pair(ap):
    """Insert a stride-0 size-2 dim after the partition dim: [p, n] ->
    [p, 2(x0), n], duplicating the data into both DoubleRow slots."""
    return bass.AP(tensor=ap.tensor, offset=ap.offset,
                   ap=[ap.ap[0], [0, 2], *ap.ap[1:]])


def build_nc(s=S, legalize=True):
    ns = s // 512  # 512-wide q slices per head
    nt = s // 128  # 128-wide s chunks

    nc = bass.Bass("TRN2", target_bir_lowering=False, debug=False)
    xh_d = nc.dram_tensor("xh", [128, 4, 2, s], F8, kind="ExternalInput").ap()
    xl_d = nc.dram_tensor("xl", [128, 4, 2, s], F8, kind="ExternalInput").ap()
    wqhl_d = nc.dram_tensor("wqhl", [128, 8, 2, GW], F8, kind="ExternalInput").ap()
    wkhl_d = nc.dram_tensor("wkhl", [128, 8, 2, GW], F8, kind="ExternalInput").ap()
    wvh_d = nc.dram_tensor("wvh", [128, 4, 2, GW], F8, kind="ExternalInput").ap()
    wvl_d = nc.dram_tensor("wvl", [128, 4, 2, GW], F8, kind="ExternalInput").ap()
    wo_d = nc.dram_tensor("wo", [128, 4, D], F16, kind="ExternalInput").ap()
    bqk_d = nc.dram_tensor("bqk", [128, 8], F32, kind="ExternalInput").ap()
    bvb_d = nc.dram_tensor("bvb", [128, GW], F16, kind="ExternalInput").ap()
    mask2_d = nc.dram_tensor("mask2", [128, 2, 128], F16, kind="ExternalInput").ap()
    out_d = nc.dram_tensor("out", [s, D], F16, kind="ExternalOutput").ap()

    with tile.TileContext(nc) as tc, ExitStack() as ctx:
        pool = lambda name, bufs, **kw: ctx.enter_context(
            tc.tile_pool(name=name, bufs=bufs, **kw)
        )
        const_p = pool("const", 1)
        x_p = pool("xp", 1)
        w_p = pool("wp", 1)
        q8_p = pool("q8p", 4)
        khl_p = pool("khlp", 4)
        v_p = pool("vp", nt)
        et_p = pool("etp", min(nt + 4, 20))
        a_p = pool("ap", 4)
        rs_p = pool("rsp", 4)
        at_p = pool("atp", 4)
        o_p = pool("op", 4)
        ps_mix = pool("psmix", 2, space="PSUM")   # proj + O-proj + transpose
        ps_qk = pool("psqk", 2, space="PSUM")     # scores [128, 2, 512]
        ps_pv = pool("pspv", 2, space="PSUM")     # PV chains [128, 65]

        # ---- input loads, ordered by first use ----
        xh_sb = x_p.tile([128, 4, 2, s], F8)
        xl_sb = x_p.tile([128, 4, 2, s], F8)
        wvh_sb = w_p.tile([128, 4, 2, GW], F8)
        wvl_sb = w_p.tile([128, 4, 2, GW], F8)
        wqhl_sb = w_p.tile([128, 8, 2, GW], F8)
        wkhl_sb = w_p.tile([128, 8, 2, GW], F8)
        wo_sb = w_p.tile([128, 4, D], F16)
        nc.sync.dma_start(
            out=xh_sb[:, 0:1, :, 0:512], in_=xh_d[:, 0:1, :, 0:512]
        )
        nc.sync.dma_start(out=wqhl_sb[:, 0:2], in_=wqhl_d[:, 0:2])
        nc.sync.dma_start(
            out=xh_sb[:, 1:4, :, 0:512], in_=xh_d[:, 1:4, :, 0:512]
        )
        nc.sync.dma_start(out=wqhl_sb[:, 2:8], in_=wqhl_d[:, 2:8])
        bqk_sb = const_p.tile([128, 8], F32)
        nc.sync.dma_start(out=bqk_sb[:], in_=bqk_d)
        nc.sync.dma_start(
            out=xh_sb[:, :, :, 512:s] if s > 512 else xh_sb[:, 0:1, :, 0:1],
            in_=xh_d[:, :, :, 512:s] if s > 512 else xh_d[:, 0:1, :, 0:1],
        )
        nc.sync.dma_start(out=wkhl_sb[:], in_=wkhl_d)
        nc.sync.dma_start(out=xl_sb[:], in_=xl_d)
        nc.sync.dma_start(out=wvh_sb[:], in_=wvh_d)
        nc.sync.dma_start(out=wvl_sb[:], in_=wvl_d)
        bvb_sb = const_p.tile([128, GW], F16)
        nc.sync.dma_start(out=bvb_sb[:], in_=bvb_d)
        mask2_sb = const_p.tile([128, 2, 128], F16)
        nc.sync.dma_start(out=mask2_sb[:], in_=mask2_d)
        nc.sync.dma_start(out=wo_sb[:], in_=wo_d)

        # touch ops: early ACT exp-table load + const observations
        scr_a = const_p.tile([128, 1], F32)
        nc.scalar.activation(scr_a[:], bqk_sb[:, 0:1], AF.Exp, scale=EXP_SCALE)
        scr_v = const_p.tile([128, 1], F16)
        nc.vector.tensor_copy(scr_v[:], bvb_sb[:, 0:1])
        scr_m = const_p.tile([128, 1], F16)
        nc.vector.tensor_copy(scr_m[:], mask2_sb[:, 0, 0:1])

        # persistent per-pair tiles
        q8_sb = [q8_p.tile([128, s], F8, tag="q8", name=f"q8_{c}") for c in range(4)]
        khl_sb = [khl_p.tile([128, nt, 256], F8, tag="khl", name=f"khl{c}")
                  for c in range(4)]
        at_sb = [at_p.tile([128, s], F16, tag="at", name=f"at{c}") for c in range(4)]
        v_sb = [None] * nt

        def v_proj(st):
            ps = ps_mix.tile([128, 512], F32, tag="ps", name="ps")
            first = True
            for (xs, ws_) in ((xh_sb, wvh_sb), (xh_sb, wvl_sb), (xl_sb, wvh_sb)):
                for cp in range(4):
                    nc.tensor.matmul(
                        ps[:],
                        xs[:, cp, :, st * 128:(st + 1) * 128],
                        ws_[:, cp],
                        start=first,
                        stop=(cp == 3 and xs is xl_sb),
                        perf_mode=DR,
                    )
                    first = False
            vt = v_p.tile([128, GH, 65], F16, tag="v", name=f"v{st}")
            nc.vector.memset(vt[:, :, 64:65], WS)
            nc.vector.tensor_add(
                vt[:, :, 0:64],
                ps[:].rearrange("p (h e) -> p h e", h=GH),
                bvb_sb[:].rearrange("p (h e) -> p h e", h=GH),
            )
            v_sb[st] = vt

        def qk_proj_slice(c, which, sl):
            whl = wqhl_sb if which == "q" else wkhl_sb
            ps = ps_mix.tile([128, 512], F32, tag="ps", name="ps")
            for dc in range(8):
                nc.tensor.matmul(
                    ps[:],
                    whl[:, dc, :, c * 128:(c + 1) * 128],
                    _stride0_pair(
                        xh_sb[:, dc // 2, dc % 2, sl * 512:(sl + 1) * 512]
                    ),
                    start=(dc == 0),
                    stop=(dc == 7),
                    perf_mode=DR,
                )
            if which == "q":
                nc.vector.tensor_scalar_add(
                    q8_sb[c][:, sl * 512:(sl + 1) * 512],
                    ps[:],
                    bqk_sb[:, c:c + 1],
                )
            else:
                kh = khl_sb[c][:, 4 * sl:4 * sl + 4, 0:128]
                kl = khl_sb[c][:, 4 * sl:4 * sl + 4, 128:256]
                psv = ps[:].rearrange("p (t m) -> p t m", t=4)
                nc.vector.tensor_scalar_add(kh, psv, bqk_sb[:, 4 + c:5 + c])
                nc.vector.scalar_tensor_tensor(
                    kl, psv, bqk_sb[:, 4 + c:5 + c], kh,
                    op0=mybir.AluOpType.add,
                    op1=mybir.AluOpType.subtract,
                )

        def o_proj(qc):
            for dsl in range(2):
                po = ps_mix.tile([128, 512], F32, tag="ps", name="po")
                for cc in range(4):
                    nc.tensor.matmul(
                        po[:],
                        at_sb[cc][:, qc * 128:(qc + 1) * 128],
                        wo_sb[:, cc, dsl * 512:(dsl + 1) * 512],
                        start=(cc == 0),
                        stop=(cc == 3),
                    )
                ob = o_p.tile([128, 512], F16, tag="ob", name="ob")
                nc.vector.tensor_copy(ob[:], po[:])
                nc.sync.dma_start(
                    out=out_d[qc * 128:(qc + 1) * 128,
                              dsl * 512:(dsl + 1) * 512],
                    in_=ob[:],
                )

        # ---- column-driven schedule ----
        # Columns (c, j) ordered so ACT-heavy late columns land next to
        # PE-heavy O-projection work; qk/V/O units drip in as PE filler.
        vq = list(range(nt))
        qkq = {cc: [(cc, w, sl) for w in ("q", "k") for sl in range(ns)]
               for cc in range(4)}
        col_done = set()
        o_emitted = set()
        tr_done = {}

        def o_ready(qc):
            return all((cc, qc // 4) in col_done for cc in range(4))

        def emit_o_ready():
            for qc in range(nt):
                if qc not in o_emitted and o_ready(qc):
                    o_emitted.add(qc)
                    return qc
            return None

        def emit_column(c, j):
            nd_t = 4 * j + 4
            et_tiles = []
            fill_credit = 0.0
            for t in range(nd_t):
                diag = t >= 4 * j
                w0 = 128 * (t - 4 * j) if diag else 0
                qk = ps_qk.tile([128, 2, 512], F32, tag="qk", name="qk")
                et = et_p.tile([128, 2, 512], F16, tag="et", name="et")
                et_tiles.append(et)
                for hh in range(2):
                    nc.tensor.matmul(
                        qk[:, hh, w0:512],
                        khl_sb[c][hh * 64:hh * 64 + 64, t].rearrange(
                            "p (i m) -> p i m", i=2
                        ),
                        _stride0_pair(
                            q8_sb[c][hh * 64:hh * 64 + 64,
                                     j * 512 + w0:(j + 1) * 512]
                        ),
                        start=True,
                        stop=True,
                        perf_mode=DR,
                    )
                nc.scalar.activation(
                    et[:, :, w0:512], qk[:, :, w0:512],
                    AF.Exp, scale=EXP_SCALE,
                )
                if diag:
                    nc.vector.tensor_mul(
                        et[:, :, w0:w0 + 128],
                        et[:, :, w0:w0 + 128],
                        mask2_sb[:],
                    )
                # keep the PE fed while ACT works through the exp backlog
                fill_credit += FILL_DIAG if diag else FILL_FULL
                while fill_credit >= 1.0:
                    if c == 0 and vq:
                        v_proj(vq.pop(0))
                        fill_credit -= 1.0
                        continue
                    nxt = None
                    for cc in range(1, 4):
                        if qkq.get(cc):
                            nxt = qkq[cc].pop(0)
                            break
                    if nxt is not None:
                        qk_proj_slice(nxt[0], nxt[1], nxt[2])
                        fill_credit -= 1.0
                        continue
                    oq = emit_o_ready()
                    if oq is not None:
                        o_proj(oq)
                        fill_credit -= 1.3
                        continue
                    break
            if c == 0:
                while vq and vq[0] <= 4 * j + 3:
                    v_proj(vq.pop(0))
            for qi in range(4):
                qc = 4 * j + qi
                a16 = a_p.tile([128, 2, 64], F16, tag="a16", name="a16")
                for hh in range(2):
                    pv = ps_pv.tile([128, 65], F32, tag="pv", name="pv")
                    for t in range(qc + 1):
                        nc.tensor.matmul(
                            pv[:],
                            et_tiles[t][:, hh, qi * 128:(qi + 1) * 128],
                            v_sb[t][:, 2 * c + hh],
                            start=(t == 0),
                            stop=(t == qc),
                        )
                    rs = rs_p.tile([128, 1], F32, tag="rs", name="rs")
                    with nc.allow_low_precision(reason="softmax divisor"):
                        nc.vector.reciprocal(rs[:], pv[:, 64:65])
                    nc.vector.tensor_scalar_mul(a16[:, hh], pv[:, 0:64], rs[:])
                nc.sync.dma_start_transpose(
                    at_sb[c][:, qc * 128:(qc + 1) * 128],
                    a16[:].rearrange("p a b -> p (a b)"),
                )
            col_done.add((c, j))

        col_order = [(0, 0), (0, 1), (0, 2), (0, 3),
                     (1, 0), (1, 1), (1, 2),
                     (2, 0), (2, 1), (2, 2),
                     (3, 0), (3, 1), (3, 2),
                     (1, 3), (3, 3), (2, 3)][:4 * ns]
        col_order = [cj for cj in col_order if cj[1] < ns]
        if ns < 4:
            col_order = [(c, j) for c in range(4) for j in range(ns)]
        for (c, j) in col_order:
            # projections this column needs: all q slices, k slices <= j
            while qkq.get(c) and qkq[c] and (
                qkq[c][0][1] == "q" or qkq[c][0][2] <= j
            ):
                it = qkq[c].pop(0)
                qk_proj_slice(it[0], it[1], it[2])
            emit_column(c, j)
        while any(qkq.values()):
            for cc in range(1, 4):
                if qkq.get(cc):
                    it = qkq[cc].pop(0)
                    qk_proj_slice(it[0], it[1], it[2])
        for qc in range(nt):
            if qc not in o_emitted:
                o_proj(qc)

    if legalize:
        _legalize_waits(nc)
    return nc


_NC_CACHE = {}


def _get_nc(s=S):
    if s not in _NC_CACHE:
        _NC_CACHE[s] = build_nc(s)
    return _NC_CACHE[s]


def _split8(x):
    hi = x.astype(NF8)
    lo = (x - hi.astype(np.float64)).astype(NF8)
    return hi, lo


def _dr_pack(w):
    """[1024 rows, cols] -> [128, 4, 2, cols] with row = (cp*2 + i)*128 + p."""
    return np.ascontiguousarray(w.reshape(4, 2, 128, -1).transpose(2, 0, 1, 3))


def _hl_pack(wh, wl):
    """(hi, lo) [1024, cols] -> [128, 8, 2, cols]: per-d-chunk (hi,lo) slots."""
    a = np.stack(
        [wh.reshape(8, 128, -1), wl.reshape(8, 128, -1)], axis=1
    )  # [dc, 2, p, m]
    return np.ascontiguousarray(a.transpose(2, 0, 1, 3))


def make_inputs(X, Wq, bq, Wk, bk, Wv, bv, Wo, bo, s=S):
    """Per-core input maps. Core c: batch c//2, head group c%2."""
    iv, jv = np.arange(128)[:, None], np.arange(128)[None, :]
    mask = (jv >= iv).astype(np.float16)
    mask2 = np.ascontiguousarray(np.broadcast_to(mask[:, None, :], (128, 2, 128)))
    X64 = X.astype(np.float64)
    in_maps = []
    splits = {}
    for g in range(2):
        lo, hi = g * GW, (g + 1) * GW
        splits[g] = {
            "wq": _split8(WS * Wq[lo:hi].T.astype(np.float64)),
            "wk": _split8(WS * Wk[lo:hi].T.astype(np.float64)),
            "wv": _split8(WS * Wv[lo:hi].T.astype(np.float64)),
            "wo": Wo[:, lo:hi].T.astype(np.float16),
            "bqk": np.concatenate(
                [
                    np.ascontiguousarray((WS * bq[lo:hi]).reshape(4, 128).T),
                    np.ascontiguousarray((WS * bk[lo:hi]).reshape(4, 128).T),
                ],
                axis=1,
            ).astype(np.float32),
            "bvb": np.tile((WS * bv[lo:hi]).astype(np.float16), (128, 1)),
        }
    xsplits = {}
    for b in range(B):
        xt = np.ascontiguousarray(X64[b, :s].T)  # [D, s]
        xsplits[b] = _split8(xt)
    for c in range(8):
        b, g = divmod(c, 2)
        sp = splits[g]
        xh, xl = xsplits[b]
        in_maps.append(
            {
                "xh": _dr_pack(xh),
                "xl": _dr_pack(xl),
                "wqhl": _hl_pack(*sp["wq"]),
                "wkhl": _hl_pack(*sp["wk"]),
                "wvh": _dr_pack(sp["wv"][0]),
                "wvl": _dr_pack(sp["wv"][1]),
                "wo": np.ascontiguousarray(
                    sp["wo"].reshape(4, 128, D).transpose(1, 0, 2)
                ),
                "bqk": sp["bqk"],
                "bvb": sp["bvb"],
                "mask2": mask2,
            }
        )
    return in_maps


def kernel(X, Wq, bq, Wk, bk, Wv, bv, Wo, bo, **run_kwargs):
    args = [np.asarray(a, np.float32) for a in (X, Wq, bq, Wk, bk, Wv, bv, Wo, bo)]
    X, Wq, bq, Wk, bk, Wv, bv, Wo, bo = args
    nc = _get_nc(S)
    in_maps = make_inputs(X, Wq, bq, Wk, bk, Wv, bv, Wo, bo, S)
    res = run_bass_kernel_spmd(nc, in_maps, core_ids=list(range(8)), **run_kwargs)
    outs = [r["out"] for r in res.results]
    full = np.empty((B, S, D), np.float32)
    for b in range(B):
        full[b] = outs[2 * b].astype(np.float32) + outs[2 * b + 1] + bo
    kernel.last_results = res
    return full


# revision 3
# speedup vs baseline: 1.0036x; 1.0036x over previous
"""Multi-head causal attention (B=4, S=2048, D=1024, H=16) on 8 NeuronCores, v2.

Sharding: core c handles batch b = c//2 and head-group g = c%2 (8 heads).
Host sums the two partial output projections per batch and adds bo.

v2 on-chip design (per core), built around the TRN2 cost model:
  - All projections run as fp8e4 DoubleRow matmuls (0.5 cyc/row, 256-deep
    contraction chunks).  Weights are host-scaled by 32 (fp8 subnormal
    avoidance) and split W = Wh + Wl; X = Xh + Xl likewise.  3-term
    products (Wh*Xh + Wl*Xh + Wh*Xl) recover ~fp16 precision at 0.75x
    the fp16 PE cost.
  - Scores: Q evacuated to fp8 (q8), K to a block (Kh|Kl) fp8 pair; one
    DoubleRow matmul per (head, block) computes q8*(Kh+Kl) via a
    stride-0 moving slot dim: 0.5x fp16 cost.
  - exp on ACT (fp32 psum -> fp16 et), scale 0.125/1024 folds the
    weight scaling out.  Causal diagonal blocks masked by a DVE
    multiply (both heads in one op).
  - PV is flipped: out [sq=128, 65] with et stationary and V moving
    (64 V cols + one 32.0 column that emits 32*rowsum, cancelling the
    32x V scale in the normalize divide).  Full 128 output partitions
    -> ~0.5x the unflipped fp16 cost.
  - Normalize: DVE tensor_scalar divide by the per-partition rowsum
    column, direct to fp16 A tiles [sq, 2*64].
  - A transposed per 128-block on the PE (fp16 identity transpose),
    evacuated by GPSIMD (Pool) to at16[g, s]; O-projection in fp16,
    Pool-evacuated to fp16 and DMA'd out.
  - Pipelining: V-projection chunks and the next pair's Q/K projections
    are interleaved into the attention phase as PE filler so the PE
    never waits on the ACT exp backlog.
  - PSUM: 8 banks exactly — qk [128,2,512]x2 (4), shared proj/O/transpose
    ring x2 (2), PV chains x2 (2).

Walrus wait-slot legality: Tile's wait assigner can emit >1 sem wait per
engine instruction; extras are split onto same-engine NoOps.
"""

import sys

for _p in ("/opt/trn_rl_repo",):
    if _p not in sys.path:
        sys.path.insert(0, _p)

from contextlib import ExitStack

import numpy as np
import ml_dtypes

import concourse.bass as bass
import concourse.mybir as mybir
import concourse.tile as tile
from concourse.bass_utils import run_bass_kernel_spmd

import bass_rust

F8 = mybir.dt.float8e4
F16 = mybir.dt.float16
F32 = mybir.dt.float32
AF = mybir.ActivationFunctionType
DR = mybir.MatmulPerfMode.DoubleRow
NF8 = ml_dtypes.float8_e4m3

B, S, D, H = 4, 2048, 1024, 16
HD = D // H  # 64
GH = 8  # heads per group
GW = GH * HD  # 512 columns per group
WS = 32.0  # host-side weight prescale (fp8 subnormal avoidance)
FILL_FULL = 0.4   # PE-filler credit per full score chunk
FILL_DIAG = 0.2   # per diagonal (windowed) chunk
ET_BUFS = 20      # et ring depth (column overlap)
EXP_SCALE = 0.125 / (WS * WS)

_SPLITTABLE = {
    "InstMatmult", "InstLdweights", "InstActivation", "InstTensorCopy",
    "InstTensorTensor", "InstTensorScalarPtr", "InstTensorReduce",
    "InstMemset", "InstDMACopy", "InstReciprocal", "InstIota",
    "InstTensorTensorReduce", "InstBNStats", "InstBNStatsAggregate",
    "InstStreamShuffle", "InstNoOp", "InstPool", "InstMax", "InstDrain",
    "InstDmaTransposeAnt",
}


def _legalize_waits(nc, max_waits=1):
    """Walrus codegen accepts at most one sync-wait command per engine
    instruction; split extras onto same-engine NoOps inserted immediately
    before (the engine blocks at the same program point)."""
    ctr = 0
    for fn in nc.m.functions:
        for blk in fn.blocks:
            out = []
            for ins in blk.instructions:
                si = ins.sync_info
                if (
                    si is not None
                    and len(si.on_wait) > max_waits
                    and type(ins).__name__ in _SPLITTABLE
                ):
                    waits = list(si.on_wait)
                    extra, keep = waits[:-max_waits], waits[-max_waits:]
                    for w in extra:
                        nop = mybir.InstNoOp(name=f"waitnop-{ctr}", ins=[], outs=[])
                        ctr += 1
                        nop.engine = ins.engine
                        nop.sync_info = bass_rust.SyncInfo(on_wait=[w], on_update=[])
                        out.append(nop)
                    ins.sync_info = bass_rust.SyncInfo(
                        on_wait=keep, on_update=list(si.on_update)
                    )
                out.append(ins)
            blk.instructions[:] = out
    return ctr


def _stride0_pair(ap):
    """Insert a stride-0 size-2 dim after the partition dim: [p, n] ->
    [p, 2(x0), n], duplicating the data into both DoubleRow slots."""
    return bass.AP(tensor=ap.tensor, offset=ap.offset,
                   ap=[ap.ap[0], [0, 2], *ap.ap[1:]])


def build_nc(s=S, legalize=True):
    ns = s // 512  # 512-wide q slices per head
    nt = s // 128  # 128-wide s chunks

    nc = bass.Bass("TRN2", target_bir_lowering=False, debug=False)
    xh_d = nc.dram_tensor("xh", [128, 4, 2, s], F8, kind="ExternalInput").ap()
    xl_d = nc.dram_tensor("xl", [128, 4, 2, s], F8, kind="ExternalInput").ap()
    wqhl_d = nc.dram_tensor("wqhl", [128, 8, 2, GW], F8, kind="ExternalInput").ap()
    wkhl_d = nc.dram_tensor("wkhl", [128, 8, 2, GW], F8, kind="ExternalInput").ap()
    wvh_d = nc.dram_tensor("wvh", [128, 4, 2, GW], F8, kind="ExternalInput").ap()
    wvl_d = nc.dram_tensor("wvl", [128, 4, 2, GW], F8, kind="ExternalInput").ap()
    wo_d = nc.dram_tensor("wo", [128, 4, D], F16, kind="ExternalInput").ap()
    bqk_d = nc.dram_tensor("bqk", [128, 8], F32, kind="ExternalInput").ap()
    bvb_d = nc.dram_tensor("bvb", [128, GW], F16, kind="ExternalInput").ap()
    mask2_d = nc.dram_tensor("mask2", [128, 2, 128], F16, kind="ExternalInput").ap()
    out_d = nc.dram_tensor("out", [s, D], F16, kind="ExternalOutput").ap()

    with tile.TileContext(nc) as tc, ExitStack() as ctx:
        pool = lambda name, bufs, **kw: ctx.enter_context(
            tc.tile_pool(name=name, bufs=bufs, **kw)
        )
        const_p = pool("const", 1)
        x_p = pool("xp", 1)
        w_p = pool("wp", 1)
        q8_p = pool("q8p", 4)
        khl_p = pool("khlp", 4)
        v_p = pool("vp", nt)
        et_p = pool("etp", min(nt + 4, ET_BUFS))
        a_p = pool("ap", 4)
        rs_p = pool("rsp", 4)
        at_p = pool("atp", 4)
        o_p = pool("op", 4)
        ps_mix = pool("psmix", 2, space="PSUM")   # proj + O-proj + transpose
        ps_qk = pool("psqk", 2, space="PSUM")     # scores [128, 2, 512]
        ps_pv = pool("pspv", 2, space="PSUM")     # PV chains [128, 65]

        # ---- input loads, ordered by first use ----
        xh_sb = x_p.tile([128, 4, 2, s], F8)
        xl_sb = x_p.tile([128, 4, 2, s], F8)
        wvh_sb = w_p.tile([128, 4, 2, GW], F8)
        wvl_sb = w_p.tile([128, 4, 2, GW], F8)
        wqhl_sb = w_p.tile([128, 8, 2, GW], F8)
        wkhl_sb = w_p.tile([128, 8, 2, GW], F8)
        wo_sb = w_p.tile([128, 4, D], F16)
        nc.sync.dma_start(
            out=xh_sb[:, 0:1, :, 0:512], in_=xh_d[:, 0:1, :, 0:512]
        )
        nc.sync.dma_start(out=wqhl_sb[:, 0:2], in_=wqhl_d[:, 0:2])
        nc.sync.dma_start(
            out=xh_sb[:, 1:4, :, 0:512], in_=xh_d[:, 1:4, :, 0:512]
        )
        nc.sync.dma_start(out=wqhl_sb[:, 2:8], in_=wqhl_d[:, 2:8])
        bqk_sb = const_p.tile([128, 8], F32)
        nc.sync.dma_start(out=bqk_sb[:], in_=bqk_d)
        nc.sync.dma_start(
            out=xh_sb[:, :, :, 512:s] if s > 512 else xh_sb[:, 0:1, :, 0:1],
            in_=xh_d[:, :, :, 512:s] if s > 512 else xh_d[:, 0:1, :, 0:1],
        )
        nc.sync.dma_start(out=wkhl_sb[:], in_=wkhl_d)
        nc.sync.dma_start(out=xl_sb[:], in_=xl_d)
        nc.sync.dma_start(out=wvh_sb[:], in_=wvh_d)
        nc.sync.dma_start(out=wvl_sb[:], in_=wvl_d)
        bvb_sb = const_p.tile([128, GW], F16)
        nc.sync.dma_start(out=bvb_sb[:], in_=bvb_d)
        mask2_sb = const_p.tile([128, 2, 128], F16)
        nc.sync.dma_start(out=mask2_sb[:], in_=mask2_d)
        nc.sync.dma_start(out=wo_sb[:], in_=wo_d)

        # touch ops: early ACT exp-table load + const observations
        scr_a = const_p.tile([128, 1], F32)
        nc.scalar.activation(scr_a[:], bqk_sb[:, 0:1], AF.Exp, scale=EXP_SCALE)
        scr_v = const_p.tile([128, 1], F16)
        nc.vector.tensor_copy(scr_v[:], bvb_sb[:, 0:1])
        scr_m = const_p.tile([128, 1], F16)
        nc.vector.tensor_copy(scr_m[:], mask2_sb[:, 0, 0:1])

        # persistent per-pair tiles
        q8_sb = [q8_p.tile([128, s], F8, tag="q8", name=f"q8_{c}") for c in range(4)]
        khl_sb = [khl_p.tile([128, nt, 256], F8, tag="khl", name=f"khl{c}")
                  for c in range(4)]
        at_sb = [at_p.tile([128, s], F16, tag="at", name=f"at{c}") for c in range(4)]
        v_sb = [None] * nt

        def v_proj(st):
            ps = ps_mix.tile([128, 512], F32, tag="ps", name="ps")
            first = True
            for (xs, ws_) in ((xh_sb, wvh_sb), (xh_sb, wvl_sb), (xl_sb, wvh_sb)):
                for cp in range(4):
                    nc.tensor.matmul(
                        ps[:],
                        xs[:, cp, :, st * 128:(st + 1) * 128],
                        ws_[:, cp],
                        start=first,
                        stop=(cp == 3 and xs is xl_sb),
                        perf_mode=DR,
                    )
                    first = False
            vt = v_p.tile([128, GH, 65], F16, tag="v", name=f"v{st}")
            nc.vector.memset(vt[:, :, 64:65], WS)
            nc.vector.tensor_add(
                vt[:, :, 0:64],
                ps[:].rearrange("p (h e) -> p h e", h=GH),
                bvb_sb[:].rearrange("p (h e) -> p h e", h=GH),
            )
            v_sb[st] = vt

        def qk_proj_slice(c, which, sl):
            whl = wqhl_sb if which == "q" else wkhl_sb
            ps = ps_mix.tile([128, 512], F32, tag="ps", name="ps")
            for dc in range(8):
                nc.tensor.matmul(
                    ps[:],
                    whl[:, dc, :, c * 128:(c + 1) * 128],
                    _stride0_pair(
                        xh_sb[:, dc // 2, dc % 2, sl * 512:(sl + 1) * 512]
                    ),
                    start=(dc == 0),
                    stop=(dc == 7),
                    perf_mode=DR,
                )
            if which == "q":
                nc.vector.tensor_scalar_add(
                    q8_sb[c][:, sl * 512:(sl + 1) * 512],
                    ps[:],
                    bqk_sb[:, c:c + 1],
                )
            else:
                kh = khl_sb[c][:, 4 * sl:4 * sl + 4, 0:128]
                kl = khl_sb[c][:, 4 * sl:4 * sl + 4, 128:256]
                psv = ps[:].rearrange("p (t m) -> p t m", t=4)
                nc.vector.tensor_scalar_add(kh, psv, bqk_sb[:, 4 + c:5 + c])
                nc.vector.scalar_tensor_tensor(
                    kl, psv, bqk_sb[:, 4 + c:5 + c], kh,
                    op0=mybir.AluOpType.add,
                    op1=mybir.AluOpType.subtract,
                )

        def o_proj(qc):
            for dsl in range(2):
                po = ps_mix.tile([128, 512], F32, tag="ps", name="po")
                for cc in range(4):
                    nc.tensor.matmul(
                        po[:],
                        at_sb[cc][:, qc * 128:(qc + 1) * 128],
                        wo_sb[:, cc, dsl * 512:(dsl + 1) * 512],
                        start=(cc == 0),
                        stop=(cc == 3),
                    )
                ob = o_p.tile([128, 512], F16, tag="ob", name="ob")
                nc.vector.tensor_copy(ob[:], po[:])
                nc.sync.dma_start(
                    out=out_d[qc * 128:(qc + 1) * 128,
                              dsl * 512:(dsl + 1) * 512],
                    in_=ob[:],
                )

        # ---- column-driven schedule ----
        # Columns (c, j) ordered so ACT-heavy late columns land next to
        # PE-heavy O-projection work; qk/V/O units drip in as PE filler.
        vq = list(range(nt))
        qkq = {cc: [(cc, w, sl) for w in ("q", "k") for sl in range(ns)]
               for cc in range(4)}
        col_done = set()
        o_emitted = set()
        tr_done = {}

        def o_ready(qc):
            return all((cc, qc // 4) in col_done for cc in range(4))

        def emit_o_ready():
            for qc in range(nt):
                if qc not in o_emitted and o_ready(qc):
                    o_emitted.add(qc)
                    return qc
            return None

        def emit_column(c, j):
            nd_t = 4 * j + 4
            et_tiles = []
            fill_credit = 0.0
            for t in range(nd_t):
                diag = t >= 4 * j
                w0 = 128 * (t - 4 * j) if diag else 0
                qk = ps_qk.tile([128, 2, 512], F32, tag="qk", name="qk")
                et = et_p.tile([128, 2, 512], F16, tag="et", name="et")
                et_tiles.append(et)
                for hh in range(2):
                    nc.tensor.matmul(
                        qk[:, hh, w0:512],
                        khl_sb[c][hh * 64:hh * 64 + 64, t].rearrange(
                            "p (i m) -> p i m", i=2
                        ),
                        _stride0_pair(
                            q8_sb[c][hh * 64:hh * 64 + 64,
                                     j * 512 + w0:(j + 1) * 512]
                        ),
                        start=True,
                        stop=True,
                        perf_mode=DR,
                    )
                nc.scalar.activation(
                    et[:, :, w0:512], qk[:, :, w0:512],
                    AF.Exp, scale=EXP_SCALE,
                )
                if diag:
                    nc.vector.tensor_mul(
                        et[:, :, w0:w0 + 128],
                        et[:, :, w0:w0 + 128],
                        mask2_sb[:],
                    )
                # keep the PE fed while ACT works through the exp backlog
                fill_credit += FILL_DIAG if diag else FILL_FULL
                while fill_credit >= 1.0:
                    if c == 0 and vq:
                        v_proj(vq.pop(0))
                        fill_credit -= 1.0
                        continue
                    nxt = None
                    for cc in range(1, 4):
                        if qkq.get(cc):
                            nxt = qkq[cc].pop(0)
                            break
                    if nxt is not None:
                        qk_proj_slice(nxt[0], nxt[1], nxt[2])
                        fill_credit -= 1.0
                        continue
                    oq = emit_o_ready()
                    if oq is not None:
                        o_proj(oq)
                        fill_credit -= 1.3
                        continue
                    break
            if c == 0:
                while vq and vq[0] <= 4 * j + 3:
                    v_proj(vq.pop(0))
            for qi in range(4):
                qc = 4 * j + qi
                a16 = a_p.tile([128, 2, 64], F16, tag="a16", name="a16")
                for hh in range(2):
                    pv = ps_pv.tile([128, 65], F32, tag="pv", name="pv")
                    for t in range(qc + 1):
                        nc.tensor.matmul(
                            pv[:],
                            et_tiles[t][:, hh, qi * 128:(qi + 1) * 128],
                            v_sb[t][:, 2 * c + hh],
                            start=(t == 0),
                            stop=(t == qc),
                        )
                    rs = rs_p.tile([128, 1], F32, tag="rs", name="rs")
                    with nc.allow_low_precision(reason="softmax divisor"):
                        nc.vector.reciprocal(rs[:], pv[:, 64:65])
                    nc.vector.tensor_scalar_mul(a16[:, hh], pv[:, 0:64], rs[:])
                nc.sync.dma_start_transpose(
                    at_sb[c][:, qc * 128:(qc + 1) * 128],
                    a16[:].rearrange("p a b -> p (a b)"),
                )
            col_done.add((c, j))

        col_order = [(0, 0), (0, 1), (0, 2), (0, 3),
                     (1, 0), (1, 1), (1, 2),
                     (2, 0), (2, 1), (3, 0),
                     (2, 2), (3, 1), (1, 3),
                     (3, 2), (2, 3), (3, 3)][:4 * ns]
        col_order = [cj for cj in col_order if cj[1] < ns]
        if ns < 4:
            col_order = [(c, j) for c in range(4) for j in range(ns)]
        for (c, j) in col_order:
            # projections this column needs: all q slices, k slices <= j
            while qkq.get(c) and qkq[c] and (
                qkq[c][0][1] == "q" or qkq[c][0][2] <= j
            ):
                it = qkq[c].pop(0)
                qk_proj_slice(it[0], it[1], it[2])
            emit_column(c, j)
        while any(qkq.values()):
            for cc in range(1, 4):
                if qkq.get(cc):
                    it = qkq[cc].pop(0)
                    qk_proj_slice(it[0], it[1], it[2])
        for qc in range(nt):
            if qc not in o_emitted:
                o_proj(qc)

    if legalize:
        _legalize_waits(nc)
    return nc


_NC_CACHE = {}


def _get_nc(s=S):
    if s not in _NC_CACHE:
        _NC_CACHE[s] = build_nc(s)
    return _NC_CACHE[s]


def _split8(x):
    hi = x.astype(NF8)
    lo = (x - hi.astype(np.float64)).astype(NF8)
    return hi, lo


def _dr_pack(w):
    """[1024 rows, cols] -> [128, 4, 2, cols] with row = (cp*2 + i)*128 + p."""
    return np.ascontiguousarray(w.reshape(4, 2, 128, -1).transpose(2, 0, 1, 3))


def _hl_pack(wh, wl):
    """(hi, lo) [1024, cols] -> [128, 8, 2, cols]: per-d-chunk (hi,lo) slots."""
    a = np.stack(
        [wh.reshape(8, 128, -1), wl.reshape(8, 128, -1)], axis=1
    )  # [dc, 2, p, m]
    return np.ascontiguousarray(a.transpose(2, 0, 1, 3))


def make_inputs(X, Wq, bq, Wk, bk, Wv, bv, Wo, bo, s=S):
    """Per-core input maps. Core c: batch c//2, head group c%2."""
    iv, jv = np.arange(128)[:, None], np.arange(128)[None, :]
    mask = (jv >= iv).astype(np.float16)
    mask2 = np.ascontiguousarray(np.broadcast_to(mask[:, None, :], (128, 2, 128)))
    X64 = X.astype(np.float64)
    in_maps = []
    splits = {}
    for g in range(2):
        lo, hi = g * GW, (g + 1) * GW
        splits[g] = {
            "wq": _split8(WS * Wq[lo:hi].T.astype(np.float64)),
            "wk": _split8(WS * Wk[lo:hi].T.astype(np.float64)),
            "wv": _split8(WS * Wv[lo:hi].T.astype(np.float64)),
            "wo": Wo[:, lo:hi].T.astype(np.float16),
            "bqk": np.concatenate(
                [
                    np.ascontiguousarray((WS * bq[lo:hi]).reshape(4, 128).T),
                    np.ascontiguousarray((WS * bk[lo:hi]).reshape(4, 128).T),
                ],
                axis=1,
            ).astype(np.float32),
            "bvb": np.tile((WS * bv[lo:hi]).astype(np.float16), (128, 1)),
        }
    xsplits = {}
    for b in range(B):
        xt = np.ascontiguousarray(X64[b, :s].T)  # [D, s]
        xsplits[b] = _split8(xt)
    for c in range(8):
        b, g = divmod(c, 2)
        sp = splits[g]
        xh, xl = xsplits[b]
        in_maps.append(
            {
                "xh": _dr_pack(xh),
                "xl": _dr_pack(xl),
                "wqhl": _hl_pack(*sp["wq"]),
                "wkhl": _hl_pack(*sp["wk"]),
                "wvh": _dr_pack(sp["wv"][0]),
                "wvl": _dr_pack(sp["wv"][1]),
                "wo": np.ascontiguousarray(
                    sp["wo"].reshape(4, 128, D).transpose(1, 0, 2)
                ),
                "bqk": sp["bqk"],
                "bvb": sp["bvb"],
                "mask2": mask2,
            }
        )
    return in_maps


def kernel(X, Wq, bq, Wk, bk, Wv, bv, Wo, bo, **run_kwargs):
    args = [np.asarray(a, np.float32) for a in (X, Wq, bq, Wk, bk, Wv, bv, Wo, bo)]
    X, Wq, bq, Wk, bk, Wv, bv, Wo, bo = args
    nc = _get_nc(S)
    in_maps = make_inputs(X, Wq, bq, Wk, bk, Wv, bv, Wo, bo, S)
    res = run_bass_kernel_spmd(nc, in_maps, core_ids=list(range(8)), **run_kwargs)
    outs = [r["out"] for r in res.results]
    full = np.empty((B, S, D), np.float32)
    for b in range(B):
        full[b] = outs[2 * b].astype(np.float32) + outs[2 * b + 1] + bo
    kernel.last_results = res
    return full


# revision 5
# speedup vs baseline: 1.0047x; 1.0011x over previous
"""Multi-head causal attention (B=4, S=2048, D=1024, H=16) on 8 NeuronCores, v2.

Sharding: core c handles batch b = c//2 and head-group g = c%2 (8 heads).
Host sums the two partial output projections per batch and adds bo.

v2 on-chip design (per core), built around the TRN2 cost model:
  - All projections run as fp8e4 DoubleRow matmuls (0.5 cyc/row, 256-deep
    contraction chunks).  Weights are host-scaled by 32 (fp8 subnormal
    avoidance) and split W = Wh + Wl; X = Xh + Xl likewise.  3-term
    products (Wh*Xh + Wl*Xh + Wh*Xl) recover ~fp16 precision at 0.75x
    the fp16 PE cost.
  - Q/K projections use a 2-term variant: (Wh,Wl) in the DoubleRow
    slots against stride-0-duplicated Xh (X quantized once to fp8).
  - Scores: Q evacuated to fp8 (q8), K to a block (Kh|Kl) fp8 pair; one
    DoubleRow matmul per (head, block) computes q8*(Kh+Kl) via a
    stride-0 moving slot dim: 0.5x fp16 cost.
  - exp on ACT (fp32 psum -> fp16 et), scale 0.125/1024 folds the
    weight scaling out.  Causal diagonal blocks masked by a DVE
    multiply (both heads in one op).
  - PV is flipped: out [sq=128, 65] with et stationary and V moving
    (64 V cols + one 32.0 column that emits 32*rowsum, cancelling the
    32x V scale in the normalize divide).  Full 128 output partitions
    -> ~0.5x the unflipped fp16 cost.
  - Normalize: DVE tensor_scalar divide by the per-partition rowsum
    column, direct to fp16 A tiles [sq, 2*64].
  - A transposed per 128-block by the DMA XBAR (dma_start_transpose)
    into at16[g, s]; O-projection in fp16, DVE-evacuated to fp16 and
    DMA'd out.
  - Scheduling: the 16 (pair, column) units run in an order that pairs
    ACT-heavy late columns with O-projection PE work; V chunks and the
    next pair's Q/K projections drip into the score loops as PE filler
    (fill-credit pacing) so the PE rarely waits on the ACT exp backlog.
  - PSUM: 8 banks — qk [128,2,512]x2 (4), proj/O ring x2 (2),
    PV chains x2 (2).

Walrus wait-slot legality: Tile's wait assigner can emit >1 sem wait per
engine instruction; extras are split onto same-engine NoOps.
"""

import sys

for _p in ("/opt/trn_rl_repo",):
    if _p not in sys.path:
        sys.path.insert(0, _p)

from contextlib import ExitStack

import numpy as np
import ml_dtypes

import concourse.bass as bass
import concourse.mybir as mybir
import concourse.tile as tile
from concourse.bass_utils import run_bass_kernel_spmd

import bass_rust

F8 = mybir.dt.float8e4
F16 = mybir.dt.float16
F32 = mybir.dt.float32
AF = mybir.ActivationFunctionType
DR = mybir.MatmulPerfMode.DoubleRow
NF8 = ml_dtypes.float8_e4m3

B, S, D, H = 4, 2048, 1024, 16
HD = D // H  # 64
GH = 8  # heads per group
GW = GH * HD  # 512 columns per group
WS = 32.0  # host-side weight prescale (fp8 subnormal avoidance)
FILL_FULL = 0.4   # PE-filler credit per full score chunk
FILL_DIAG = 0.2   # per diagonal (windowed) chunk
ET_BUFS = 20      # et ring depth (column overlap)
O_RESERVE = 0     # O-proj units withheld for the final columns
LAST_COLS = {(2, 3), (3, 3)}
LAST_BOOST = 1.0
EXP_SCALE = 0.125 / (WS * WS)

_SPLITTABLE = {
    "InstMatmult", "InstLdweights", "InstActivation", "InstTensorCopy",
    "InstTensorTensor", "InstTensorScalarPtr", "InstTensorReduce",
    "InstMemset", "InstDMACopy", "InstReciprocal", "InstIota",
    "InstTensorTensorReduce", "InstBNStats", "InstBNStatsAggregate",
    "InstStreamShuffle", "InstNoOp", "InstPool", "InstMax", "InstDrain",
    "InstDmaTransposeAnt",
}


def _legalize_waits(nc, max_waits=1):
    """Walrus codegen accepts at most one sync-wait command per engine
    instruction; split extras onto same-engine NoOps inserted immediately
    before (the engine blocks at the same program point)."""
    ctr = 0
    for fn in nc.m.functions:
        for blk in fn.blocks:
            out = []
            for ins in blk.instructions:
                si = ins.sync_info
                if (
                    si is not None
                    and len(si.on_wait) > max_waits
                    and type(ins).__name__ in _SPLITTABLE
                ):
                    waits = list(si.on_wait)
                    extra, keep = waits[:-max_waits], waits[-max_waits:]
                    for w in extra:
                        nop = mybir.InstNoOp(name=f"waitnop-{ctr}", ins=[], outs=[])
                        ctr += 1
                        nop.engine = ins.engine
                        nop.sync_info = bass_rust.SyncInfo(on_wait=[w], on_update=[])
                        out.append(nop)
                    ins.sync_info = bass_rust.SyncInfo(
                        on_wait=keep, on_update=list(si.on_update)
                    )
                out.append(ins)
            blk.instructions[:] = out
    return ctr


def _stride0_pair(ap):
    """Insert a stride-0 size-2 dim after the partition dim: [p, n] ->
    [p, 2(x0), n], duplicating the data into both DoubleRow slots."""
    return bass.AP(tensor=ap.tensor, offset=ap.offset,
                   ap=[ap.ap[0], [0, 2], *ap.ap[1:]])


def build_nc(s=S, legalize=True):
    ns = s // 512  # 512-wide q slices per head
    nt = s // 128  # 128-wide s chunks

    nc = bass.Bass("TRN2", target_bir_lowering=False, debug=False)
    xh_d = nc.dram_tensor("xh", [128, 4, 2, s], F8, kind="ExternalInput").ap()
    xl_d = nc.dram_tensor("xl", [128, 4, 2, s], F8, kind="ExternalInput").ap()
    wqhl_d = nc.dram_tensor("wqhl", [128, 4, 8, 2, 128], F8, kind="ExternalInput").ap()
    wkhl_d = nc.dram_tensor("wkhl", [128, 4, 8, 2, 128], F8, kind="ExternalInput").ap()
    wvh_d = nc.dram_tensor("wvh", [128, 4, 2, GW], F8, kind="ExternalInput").ap()
    wvl_d = nc.dram_tensor("wvl", [128, 4, 2, GW], F8, kind="ExternalInput").ap()
    wo_d = nc.dram_tensor("wo", [128, 4, D], F16, kind="ExternalInput").ap()
    bqk_d = nc.dram_tensor("bqk", [128, 8], F32, kind="ExternalInput").ap()
    bvb_d = nc.dram_tensor("bvb", [128, GW], F16, kind="ExternalInput").ap()
    mask2_d = nc.dram_tensor("mask2", [128, 2, 128], F16, kind="ExternalInput").ap()
    out_d = nc.dram_tensor("out", [s, D], F16, kind="ExternalOutput").ap()

    with tile.TileContext(nc) as tc, ExitStack() as ctx:
        pool = lambda name, bufs, **kw: ctx.enter_context(
            tc.tile_pool(name=name, bufs=bufs, **kw)
        )
        const_p = pool("const", 1)
        x_p = pool("xp", 1)
        w_p = pool("wp", 1)
        q8_p = pool("q8p", 4)
        khl_p = pool("khlp", 4)
        v_p = pool("vp", nt)
        et_p = pool("etp", min(nt + 4, ET_BUFS))
        a_p = pool("ap", 4)
        rs_p = pool("rsp", 4)
        at_p = pool("atp", 4)
        o_p = pool("op", 4)
        ps_mix = pool("psmix", 2, space="PSUM")   # proj + O-proj + transpose
        ps_qk = pool("psqk", 2, space="PSUM")     # scores [128, 2, 512]
        ps_pv = pool("pspv", 2, space="PSUM")     # PV chains [128, 65]

        # ---- input loads, ordered by first use ----
        xh_sb = x_p.tile([128, 4, 2, s], F8)
        xl_sb = x_p.tile([128, 4, 2, s], F8)
        wvh_sb = w_p.tile([128, 4, 2, GW], F8)
        wvl_sb = w_p.tile([128, 4, 2, GW], F8)
        wqhl_sb = w_p.tile([128, 4, 8, 2, 128], F8)
        wkhl_sb = w_p.tile([128, 4, 8, 2, 128], F8)
        wo_sb = w_p.tile([128, 4, D], F16)
        nc.sync.dma_start(
            out=xh_sb[:, :, :, 0:512], in_=xh_d[:, :, :, 0:512]
        )
        nc.sync.dma_start(out=wqhl_sb[:, 0], in_=wqhl_d[:, 0])
        nc.sync.dma_start(out=wkhl_sb[:, 0], in_=wkhl_d[:, 0])
        bqk_sb = const_p.tile([128, 8], F32)
        nc.sync.dma_start(out=bqk_sb[:], in_=bqk_d)
        nc.sync.dma_start(
            out=xh_sb[:, :, :, 512:s] if s > 512 else xh_sb[:, 0:1, :, 0:1],
            in_=xh_d[:, :, :, 512:s] if s > 512 else xh_d[:, 0:1, :, 0:1],
        )
        nc.sync.dma_start(out=wqhl_sb[:, 1:4], in_=wqhl_d[:, 1:4])
        nc.sync.dma_start(out=wkhl_sb[:, 1:4], in_=wkhl_d[:, 1:4])
        nc.sync.dma_start(out=xl_sb[:], in_=xl_d)
        nc.sync.dma_start(out=wvh_sb[:], in_=wvh_d)
        nc.sync.dma_start(out=wvl_sb[:], in_=wvl_d)
        bvb_sb = const_p.tile([128, GW], F16)
        nc.sync.dma_start(out=bvb_sb[:], in_=bvb_d)
        mask2_sb = const_p.tile([128, 2, 128], F16)
        nc.sync.dma_start(out=mask2_sb[:], in_=mask2_d)
        nc.sync.dma_start(out=wo_sb[:], in_=wo_d)

        # touch ops: early ACT exp-table load + const observations
        scr_a = const_p.tile([128, 1], F32)
        nc.scalar.activation(scr_a[:], bqk_sb[:, 0:1], AF.Exp, scale=EXP_SCALE)
        scr_v = const_p.tile([128, 1], F16)
        nc.vector.tensor_copy(scr_v[:], bvb_sb[:, 0:1])
        scr_m = const_p.tile([128, 1], F16)
        nc.vector.tensor_copy(scr_m[:], mask2_sb[:, 0, 0:1])

        # persistent per-pair tiles
        q8_sb = [q8_p.tile([128, s], F8, tag="q8", name=f"q8_{c}") for c in range(4)]
        khl_sb = [khl_p.tile([128, nt, 256], F8, tag="khl", name=f"khl{c}")
                  for c in range(4)]
        at_sb = [at_p.tile([128, s], F16, tag="at", name=f"at{c}") for c in range(4)]
        v_sb = [None] * nt

        def v_proj(st):
            ps = ps_mix.tile([128, 512], F32, tag="ps", name="ps")
            first = True
            for (xs, ws_) in ((xh_sb, wvh_sb), (xh_sb, wvl_sb), (xl_sb, wvh_sb)):
                for cp in range(4):
                    nc.tensor.matmul(
                        ps[:],
                        xs[:, cp, :, st * 128:(st + 1) * 128],
                        ws_[:, cp],
                        start=first,
                        stop=(cp == 3 and xs is xl_sb),
                        perf_mode=DR,
                    )
                    first = False
            vt = v_p.tile([128, GH, 65], F16, tag="v", name=f"v{st}")
            nc.vector.memset(vt[:, :, 64:65], WS)
            nc.vector.tensor_add(
                vt[:, :, 0:64],
                ps[:].rearrange("p (h e) -> p h e", h=GH),
                bvb_sb[:].rearrange("p (h e) -> p h e", h=GH),
            )
            v_sb[st] = vt

        def qk_proj_slice(c, which, sl):
            whl = wqhl_sb if which == "q" else wkhl_sb
            ps = ps_mix.tile([128, 512], F32, tag="ps", name="ps")
            for dc in range(8):
                nc.tensor.matmul(
                    ps[:],
                    whl[:, c, dc],
                    _stride0_pair(
                        xh_sb[:, dc // 2, dc % 2, sl * 512:(sl + 1) * 512]
                    ),
                    start=(dc == 0),
                    stop=(dc == 7),
                    perf_mode=DR,
                )
            if which == "q":
                nc.vector.tensor_scalar_add(
                    q8_sb[c][:, sl * 512:(sl + 1) * 512],
                    ps[:],
                    bqk_sb[:, c:c + 1],
                )
            else:
                kh = khl_sb[c][:, 4 * sl:4 * sl + 4, 0:128]
                kl = khl_sb[c][:, 4 * sl:4 * sl + 4, 128:256]
                psv = ps[:].rearrange("p (t m) -> p t m", t=4)
                nc.vector.tensor_scalar_add(kh, psv, bqk_sb[:, 4 + c:5 + c])
                nc.vector.scalar_tensor_tensor(
                    kl, psv, bqk_sb[:, 4 + c:5 + c], kh,
                    op0=mybir.AluOpType.add,
                    op1=mybir.AluOpType.subtract,
                )

        def o_proj(qc):
            for dsl in range(2):
                po = ps_mix.tile([128, 512], F32, tag="ps", name="po")
                for cc in range(4):
                    nc.tensor.matmul(
                        po[:],
                        at_sb[cc][:, qc * 128:(qc + 1) * 128],
                        wo_sb[:, cc, dsl * 512:(dsl + 1) * 512],
                        start=(cc == 0),
                        stop=(cc == 3),
                    )
                ob = o_p.tile([128, 512], F16, tag="ob", name="ob")
                nc.vector.tensor_copy(ob[:], po[:])
                nc.sync.dma_start(
                    out=out_d[qc * 128:(qc + 1) * 128,
                              dsl * 512:(dsl + 1) * 512],
                    in_=ob[:],
                )

        # ---- column-driven schedule ----
        # Columns (c, j) ordered so ACT-heavy late columns land next to
        # PE-heavy O-projection work; qk/V/O units drip in as PE filler.
        vq = list(range(nt))
        qkq = {cc: [(cc, w, sl) for w in ("q", "k") for sl in range(ns)]
               for cc in range(4)}
        col_done = set()
        o_emitted = set()
        tr_done = {}

        def o_ready(qc):
            return all((cc, qc // 4) in col_done for cc in range(4))

        def emit_o_ready(reserve=0):
            avail = [qc for qc in range(nt)
                     if qc not in o_emitted and o_ready(qc)]
            if len(avail) <= reserve:
                return None
            o_emitted.add(avail[0])
            return avail[0]

        def emit_column(c, j):
            nd_t = 4 * j + 4
            et_tiles = []
            fill_credit = 0.0
            for t in range(nd_t):
                diag = t >= 4 * j
                w0 = 128 * (t - 4 * j) if diag else 0
                qk = ps_qk.tile([128, 2, 512], F32, tag="qk", name="qk")
                et = et_p.tile([128, 2, 512], F16, tag="et", name="et")
                et_tiles.append(et)
                for hh in range(2):
                    nc.tensor.matmul(
                        qk[:, hh, w0:512],
                        khl_sb[c][hh * 64:hh * 64 + 64, t].rearrange(
                            "p (i m) -> p i m", i=2
                        ),
                        _stride0_pair(
                            q8_sb[c][hh * 64:hh * 64 + 64,
                                     j * 512 + w0:(j + 1) * 512]
                        ),
                        start=True,
                        stop=True,
                        perf_mode=DR,
                    )
                nc.scalar.activation(
                    et[:, :, w0:512], qk[:, :, w0:512],
                    AF.Exp, scale=EXP_SCALE,
                )
                if diag:
                    nc.vector.tensor_mul(
                        et[:, :, w0:w0 + 128],
                        et[:, :, w0:w0 + 128],
                        mask2_sb[:],
                    )
                # keep the PE fed while ACT works through the exp backlog
                boost = LAST_BOOST if (c, j) in LAST_COLS else 1.0
                fill_credit += (FILL_DIAG if diag else FILL_FULL) * boost
                while fill_credit >= 1.0:
                    if c == 0 and vq:
                        v_proj(vq.pop(0))
                        fill_credit -= 1.0
                        continue
                    nxt = None
                    for cc in range(1, 4):
                        if qkq.get(cc):
                            nxt = qkq[cc].pop(0)
                            break
                    if nxt is not None:
                        qk_proj_slice(nxt[0], nxt[1], nxt[2])
                        fill_credit -= 1.0
                        continue
                    oq = emit_o_ready(O_RESERVE if (c, j) not in LAST_COLS
                                      else 0)
                    if oq is not None:
                        o_proj(oq)
                        fill_credit -= 1.3
                        continue
                    break
            if c == 0:
                while vq and vq[0] <= 4 * j + 3:
                    v_proj(vq.pop(0))
            for qi in range(4):
                qc = 4 * j + qi
                a16 = a_p.tile([128, 2, 64], F16, tag="a16", name="a16")
                for hh in range(2):
                    pv = ps_pv.tile([128, 65], F32, tag="pv", name="pv")
                    for t in range(qc + 1):
                        nc.tensor.matmul(
                            pv[:],
                            et_tiles[t][:, hh, qi * 128:(qi + 1) * 128],
                            v_sb[t][:, 2 * c + hh],
                            start=(t == 0),
                            stop=(t == qc),
                        )
                    rs = rs_p.tile([128, 1], F32, tag="rs", name="rs")
                    with nc.allow_low_precision(reason="softmax divisor"):
                        nc.vector.reciprocal(rs[:], pv[:, 64:65])
                    nc.vector.tensor_scalar_mul(a16[:, hh], pv[:, 0:64], rs[:])
                nc.sync.dma_start_transpose(
                    at_sb[c][:, qc * 128:(qc + 1) * 128],
                    a16[:].rearrange("p a b -> p (a b)"),
                )
            col_done.add((c, j))

        col_order = [(0, 0), (0, 1), (0, 2), (0, 3),
                     (1, 0), (1, 1), (1, 2),
                     (2, 0), (2, 1), (3, 0),
                     (2, 2), (3, 1), (1, 3),
                     (3, 2), (2, 3), (3, 3)][:4 * ns]
        col_order = [cj for cj in col_order if cj[1] < ns]
        if ns < 4:
            col_order = [(c, j) for c in range(4) for j in range(ns)]
        for (c, j) in col_order:
            # projections this column needs: all q slices, k slices <= j
            while qkq.get(c) and qkq[c] and (
                qkq[c][0][1] == "q" or qkq[c][0][2] <= j
            ):
                it = qkq[c].pop(0)
                qk_proj_slice(it[0], it[1], it[2])
            emit_column(c, j)
        while any(qkq.values()):
            for cc in range(1, 4):
                if qkq.get(cc):
                    it = qkq[cc].pop(0)
                    qk_proj_slice(it[0], it[1], it[2])
        for qc in range(nt):
            if qc not in o_emitted:
                o_proj(qc)

    if legalize:
        _legalize_waits(nc)
    return nc


_NC_CACHE = {}


def _get_nc(s=S):
    if s not in _NC_CACHE:
        _NC_CACHE[s] = build_nc(s)
    return _NC_CACHE[s]


def _split8(x):
    hi = x.astype(NF8)
    lo = (x - hi.astype(np.float64)).astype(NF8)
    return hi, lo


def _dr_pack(w):
    """[1024 rows, cols] -> [128, 4, 2, cols] with row = (cp*2 + i)*128 + p."""
    return np.ascontiguousarray(w.reshape(4, 2, 128, -1).transpose(2, 0, 1, 3))


def _hl_pack(wh, wl):
    """(hi, lo) [1024, 512] -> [128, 4(c), 8(dc), 2, 128]: c-major with
    per-d-chunk (hi,lo) DoubleRow slots."""
    a = np.stack(
        [wh.reshape(8, 128, 4, 128), wl.reshape(8, 128, 4, 128)], axis=1
    )  # [dc, 2, p, c, m]
    return np.ascontiguousarray(a.transpose(2, 3, 0, 1, 4))


def make_inputs(X, Wq, bq, Wk, bk, Wv, bv, Wo, bo, s=S):
    """Per-core input maps. Core c: batch c//2, head group c%2."""
    iv, jv = np.arange(128)[:, None], np.arange(128)[None, :]
    mask = (jv >= iv).astype(np.float16)
    mask2 = np.ascontiguousarray(np.broadcast_to(mask[:, None, :], (128, 2, 128)))
    X64 = X.astype(np.float64)
    in_maps = []
    splits = {}
    for g in range(2):
        lo, hi = g * GW, (g + 1) * GW
        splits[g] = {
            "wq": _split8(WS * Wq[lo:hi].T.astype(np.float64)),
            "wk": _split8(WS * Wk[lo:hi].T.astype(np.float64)),
            "wv": _split8(WS * Wv[lo:hi].T.astype(np.float64)),
            "wo": Wo[:, lo:hi].T.astype(np.float16),
            "bqk": np.concatenate(
                [
                    np.ascontiguousarray((WS * bq[lo:hi]).reshape(4, 128).T),
                    np.ascontiguousarray((WS * bk[lo:hi]).reshape(4, 128).T),
                ],
                axis=1,
            ).astype(np.float32),
            "bvb": np.tile((WS * bv[lo:hi]).astype(np.float16), (128, 1)),
        }
    xsplits = {}
    for b in range(B):
        xt = np.ascontiguousarray(X64[b, :s].T)  # [D, s]
        xsplits[b] = _split8(xt)
    for c in range(8):
        b, g = divmod(c, 2)
        sp = splits[g]
        xh, xl = xsplits[b]
        in_maps.append(
            {
                "xh": _dr_pack(xh),
                "xl": _dr_pack(xl),
                "wqhl": _hl_pack(*sp["wq"]),
                "wkhl": _hl_pack(*sp["wk"]),
                "wvh": _dr_pack(sp["wv"][0]),
                "wvl": _dr_pack(sp["wv"][1]),
                "wo": np.ascontiguousarray(
                    sp["wo"].reshape(4, 128, D).transpose(1, 0, 2)
                ),
                "bqk": sp["bqk"],
                "bvb": sp["bvb"],
                "mask2": mask2,
            }
        )
    return in_maps


def kernel(X, Wq, bq, Wk, bk, Wv, bv, Wo, bo, **run_kwargs):
    args = [np.asarray(a, np.float32) for a in (X, Wq, bq, Wk, bk, Wv, bv, Wo, bo)]
    X, Wq, bq, Wk, bk, Wv, bv, Wo, bo = args
    nc = _get_nc(S)
    in_maps = make_inputs(X, Wq, bq, Wk, bk, Wv, bv, Wo, bo, S)
    res = run_bass_kernel_spmd(nc, in_maps, core_ids=list(range(8)), **run_kwargs)
    outs = [r["out"] for r in res.results]
    full = np.empty((B, S, D), np.float32)
    for b in range(B):
        full[b] = outs[2 * b].astype(np.float32) + outs[2 * b + 1] + bo
    kernel.last_results = res
    return full
